# revision 27
# baseline (speedup 1.0000x reference)
"""Trainium2 Bass kernel for 3-layer GAT + BN/ReLU + global max pool + FC.

Sharding: nodes renumbered so each of the 16 graphs (batch segments) gets a
fixed padded block of SEGLEN nodes; core k owns segments {2k,2k+1} (NLOC
contiguous device nodes). Edges partitioned by destination core, grouped in
128-dst windows; segmented softmax-sum + feature aggregation are done as PE
matmuls against on-device-built 0/1 selector matrices. Per-edge source rows
come from one dma_gather per window out of a per-layer DRAM table
[h bf16 (c-major) | als bf16 | ald bf16 | pad].

v2: dense phase distributed — each core computes h only for its own NLOC
rows and an AllGather with a Shared-HBM output materializes the full table
once per core pair; BN stats travel in a separate tiny AllGather so the
activation+dense for the next layer runs on local data only. Feature
channels are stored c-major (c*H+h) so the per-edge alpha*h multiply hits
the DVE 2x mode; per-subchunk denominator columns ride in the same matmul
as the feature aggregation for layers 1-2.
"""
import sys

sys.path.insert(0, "/opt/trn_rl_repo")
sys.path.insert(0, "/opt/trn_rl_repo/concourse")

import numpy as np
import ml_dtypes

import concourse.bass as bass
import concourse.tile as tile
import concourse.mybir as mybir
from concourse import bacc
from concourse.bass import IndirectOffsetOnAxis

P = 128
NCORES = 8
H = 8
EPS_BN = 1e-5
NEG_SLOPE = 0.2
F32 = mybir.dt.float32
BF16 = mybir.dt.bfloat16

CINS = [3, 128, 256]
CS = [16, 32, 64]
HCS = [128, 256, 512]
ROWES = [256, 384, 640]  # table row width in bf16 elems (256B multiples)
NTILES = [1, 2, 4]       # HC/128 per layer
NKS = [1, 1, 2]          # K-tiles per dense matmul

AX = mybir.AxisListType
ALU = mybir.AluOpType
ACTF = mybir.ActivationFunctionType


def apx(sl, dims):
    """AP with custom free dims [(step, count), ...] on a tile slice's base."""
    return bass.AP(sl.tensor, sl.offset,
                   [list(sl.ap[0])] + [list(d) for d in dims])


def hoist_excess_waits(nc, cap=1):
    cnt = [0]
    total = 0
    for f in nc.m.functions:
        for b in f.blocks:
            if not any(
                i.sync_info is not None and i.sync_info.on_wait
                and len(i.sync_info.on_wait) > cap
                for i in b.instructions
            ):
                continue
            new_insts = []
            for inst in b.instructions:
                si = inst.sync_info
                if si is not None and si.on_wait and len(si.on_wait) > cap:
                    waits = list(si.on_wait)
                    for w in waits[:-cap]:
                        cnt[0] += 1
                        new_insts.append(
                            mybir.InstNoOp(
                                name=f"hw-{cnt[0]}",
                                engine=inst.engine,
                                ins=[],
                                outs=[],
                                sync_info=mybir.SyncInfo(on_wait=[w],
                                                         on_update=[]),
                            )
                        )
                        total += 1
                    inst.sync_info = mybir.SyncInfo(
                        on_wait=waits[-cap:], on_update=list(si.on_update))
                new_insts.append(inst)
            b.instructions = new_insts
    return total


def patch_walrus():
    from concourse import bass_utils as bu
    if getattr(bu, "_dge_patched", False):
        return
    orig = bu.get_walrus_args

    def patched(*a, **k):
        return orig(*a, **k) + [
            "--dge-levels=io,spill_reload,scalar_dynamic_offset,"
            "vector_dynamic_offsets,dst_reduce"
        ]

    bu.get_walrus_args = patched
    bu._dge_patched = True


def build_gnn(SEGLEN, NSUB, n_real, stages=99, repeat=1):
    nsubs = list(NSUB) if isinstance(NSUB, (list, tuple)) else None
    NLOC = 2 * SEGLEN
    NW = NLOC // P
    NPD = 16 * SEGLEN
    SEGW = NW // 2
    if nsubs is None:
        nsubs = [NSUB] * NW
    csum = [0]
    for v in nsubs:
        csum.append(csum[-1] + v)
    TS = csum[-1]  # total subchunks per core

    nc = bacc.Bacc("TRN2", target_bir_lowering=False, debug=False,
                   num_devices=NCORES, num_swdge_queues=4)

    # ---------------- I/O ----------------
    xT = nc.dram_tensor("xT", [3, NLOC], BF16, kind="ExternalInput")
    wc = [nc.dram_tensor(f"wc{l}", [P if l > 1 else 3,
                                    NKS[l - 1] * (HCS[l - 1] + 16)], BF16,
                         kind="ExternalInput") for l in (1, 2, 3)]
    bng = [nc.dram_tensor(f"bng{l}", [P, NTILES[l - 1]], F32,
                          kind="ExternalInput") for l in (1, 2, 3)]
    bnb = [nc.dram_tensor(f"bnb{l}", [P, NTILES[l - 1]], F32,
                          kind="ExternalInput") for l in (1, 2, 3)]
    fcwb = nc.dram_tensor("fcwb", [513, 10], F32, kind="ExternalInput")
    srcs = nc.dram_tensor("srcs", [P, TS * 8], mybir.dt.int16,
                          kind="ExternalInput")
    dstloc = nc.dram_tensor("dstloc", [P, TS], BF16,
                            kind="ExternalInput")
    dstrep = nc.dram_tensor("dstrep", [P, TS * P], BF16,
                            kind="ExternalInput")
    dstids = nc.dram_tensor("dstids", [P, NW], mybir.dt.int32,
                            kind="ExternalInput")
    pmask = nc.dram_tensor("pmask", [1, NLOC], F32, kind="ExternalInput")
    ioro = nc.dram_tensor("ioro", [P, P], BF16, kind="ExternalInput")
    ioco = nc.dram_tensor("ioco", [P, 1], BF16, kind="ExternalInput")
    iocr = nc.dram_tensor("iocr", [P, max(nsubs) * P], BF16,
                          kind="ExternalInput")
    idnt = nc.dram_tensor("idnt", [P, P], F32, kind="ExternalInput")
    out = nc.dram_tensor("out", [16, 10], F32, kind="ExternalOutput")

    # ---------------- internal DRAM ----------------
    tables = [nc.dram_tensor(f"table{l}", [NPD, ROWES[l - 1]], BF16,
                             addr_space="Shared")
              for l in (1, 2, 3)]
    ag_in = [nc.dram_tensor(f"ag{l}_in", [NLOC, ROWES[l - 1]], BF16)
             for l in (1, 2, 3)]
    st_in = [nc.dram_tensor(f"st{l}_in", [P, 4 * NTILES[l - 1]], BF16)
             for l in (1, 2)]
    st_out = [nc.dram_tensor(f"st{l}_out", [NCORES * P, 4 * NTILES[l - 1]],
                             BF16) for l in (1, 2)]
    st3_in = nc.dram_tensor("st3_in", [4 * P, 4], F32)
    st3_out = nc.dram_tensor("st3_out", [NCORES * 4 * P, 4], F32)

    RG = [list(range(NCORES))]
    WG = 6  # windows per batched ag_in row write

    with tile.TileContext(nc) as tc, tc.tile_pool(name="const", bufs=1) as cp:
        # ---------- constants ----------
        iota_row = cp.tile([P, P], BF16, tag="ioro", name="ioro")
        nc.sync.dma_start(iota_row[:], ioro.ap())
        iota_col = cp.tile([P, 1], BF16, tag="ioco", name="ioco")
        nc.sync.dma_start(iota_col[:], ioco.ap())
        icr = cp.tile([P, max(nsubs) * P], BF16, tag="iocr", name="iocr")
        nc.sync.dma_start(icr[:], iocr.ap())
        ident = cp.tile([P, P], F32, tag="idnt", name="idnt")
        nc.sync.dma_start(ident[:], idnt.ap())
        onesf = cp.tile([1, P], F32, tag="onesf", name="onesf")
        nc.vector.memset(onesf[:], 1.0)
        ones16 = cp.tile([1, 16], F32, tag="ones16", name="ones16")
        nc.vector.memset(ones16[:], 1.0)

        xT_t = cp.tile([3, NLOC], BF16, tag="xT", name="xT")
        nc.sync.dma_start(xT_t[:], xT.ap())
        wc_t = []
        for i in range(3):
            t = cp.tile([P if i > 0 else 3, NKS[i] * (HCS[i] + 16)], BF16,
                        tag=f"wc{i}", name=f"wc{i}")
            nc.sync.dma_start(t[:], wc[i].ap())
            wc_t.append(t)
        bng_t, bnb_t = [], []
        for i in range(3):
            tg = cp.tile([P, NTILES[i]], F32, tag=f"bng{i}", name=f"bng{i}")
            tb = cp.tile([P, NTILES[i]], F32, tag=f"bnb{i}", name=f"bnb{i}")
            nc.sync.dma_start(tg[:], bng[i].ap())
            nc.sync.dma_start(tb[:], bnb[i].ap())
            bng_t.append(tg)
            bnb_t.append(tb)
        fcw_t = []
        for c in range(4):
            t = cp.tile([P, 10], F32, tag=f"fcw{c}", name=f"fcw{c}")
            nc.sync.dma_start(t[:], fcwb.ap()[c * P:(c + 1) * P, :])
            fcw_t.append(t)
        fcb_t = cp.tile([1, 10], F32, tag="fcb", name="fcb")
        nc.sync.dma_start(fcb_t[:], fcwb.ap()[512:513, :])
        pmask_t = cp.tile([1, NLOC], F32, tag="pmask", name="pmask")
        nc.sync.dma_start(pmask_t[:], pmask.ap())
        idx_t = cp.tile([P, TS * 8], mybir.dt.int16, tag="idx", name="idx")
        nc.sync.dma_start(idx_t[:], srcs.ap())
        dloc_t = cp.tile([P, TS], BF16, tag="dloc", name="dloc")
        nc.sync.dma_start(dloc_t[:], dstloc.ap())
        dids_t = cp.tile([P, NW], mybir.dt.int32, tag="dids", name="dids")
        nc.sync.dma_start(dids_t[:], dstids.ap())

        # own windows' ald values, stashed by each dense phase for the edge
        # phase (avoids an indirect table read-back per window)
        aldall = cp.tile([P, NW, 8], BF16, tag="aldall", name="aldall")

        # persistent edge->dst selector: sel_all[p, t, j] = (dl[t*P+p] == j),
        # built once, reused as psf/psd lhsT by every layer and repeat.
        sel_all = cp.tile([P, TS, P], BF16, tag="sel_all", name="sel_all")
        for w in range(NW):
            ns = nsubs[w]
            base = csum[w]
            nc.vector.tensor_tensor(
                out=sel_all[:, base:base + ns, :],
                in0=apx(dloc_t[:, base:base + ns], [(1, ns), (0, P)]),
                in1=apx(iota_row[:], [(0, ns), (1, P)]),
                op=ALU.is_equal)

        def compute_AB(pool, s12, gi, c):
            mu = pool.tile([P, 1], F32, tag="mu", name="mu")
            nc.vector.tensor_scalar(mu[:], s12[:, 0:1], 1.0 / n_real, None,
                                    op0=ALU.mult)
            ex2 = pool.tile([P, 1], F32, tag="ex2", name="ex2")
            nc.vector.tensor_scalar(ex2[:], s12[:, 1:2], 1.0 / n_real, None,
                                    op0=ALU.mult)
            var = pool.tile([P, 1], F32, tag="var", name="var")
            nc.vector.tensor_tensor(out=var[:], in0=mu[:], in1=mu[:],
                                    op=ALU.mult)
            nc.vector.tensor_tensor(out=var[:], in0=ex2[:], in1=var[:],
                                    op=ALU.subtract)
            nc.vector.tensor_scalar(var[:], var[:], EPS_BN, None, op0=ALU.add)
            sd = pool.tile([P, 1], F32, tag="sd", name="sd")
            nc.scalar.sqrt(sd[:], var[:])
            rs = pool.tile([P, 1], F32, tag="rs", name="rs")
            nc.vector.reciprocal(rs[:], sd[:])
            A = pool.tile([P, 1], F32, tag="A", name="A")
            nc.vector.tensor_tensor(out=A[:], in0=rs[:],
                                    in1=bng_t[gi][:, c:c + 1], op=ALU.mult)
            B = pool.tile([P, 1], F32, tag="B", name="B")
            nc.vector.tensor_tensor(out=B[:], in0=mu[:], in1=A[:],
                                    op=ALU.mult)
            nc.vector.tensor_tensor(out=B[:], in0=bnb_t[gi][:, c:c + 1],
                                    in1=B[:], op=ALU.subtract)
            return A, B

        def dense_phase(l, zt_tiles):
            """Own-rows dense: h = x @ [W|As|Ad] for this core's NLOC nodes,
            rows written (batched) to ag_in[l], then AllGather -> Shared
            table. For l>1 the input x is built inline from zt_tiles (pre-BN
            z, own nodes) using BN stats from the st AllGather. Own windows'
            ald columns are stashed f32-exact in aldall for the edge phase."""
            li = l - 1
            HC, ROWE = HCS[li], ROWES[li]
            nk = NKS[li]
            WCW = HC + 16
            with (
                tc.tile_pool(name=f"d{l}", bufs=2) as dp,
                tc.tile_pool(name=f"dx{l}", bufs=1) as xp,
                tc.tile_pool(name=f"dps{l}", bufs=3, space="PSUM") as pp,
            ):
                x_tiles = []
                if l > 1:
                    ntile_in = NTILES[li - 1]
                    # global BN stats: sum hi/lo pairs from all cores
                    for c in range(ntile_in):
                        acc = dp.tile([P, 2], F32, tag="acc", name="acc")
                        nc.vector.memset(acc[:], 0.0)
                        for k in range(NCORES):
                            st = dp.tile([P, 4], BF16, tag="st", name="st")
                            nc.sync.dma_start(
                                st[:],
                                st_out[li - 1].ap()[k * P:(k + 1) * P,
                                                    4 * c:4 * (c + 1)])
                            nc.vector.tensor_tensor(
                                out=acc[:], in0=acc[:], in1=st[:, 0:2],
                                op=ALU.add)
                            nc.vector.tensor_tensor(
                                out=acc[:], in0=acc[:], in1=st[:, 2:4],
                                op=ALU.add)
                        A, B = compute_AB(dp, acc, li - 1, c)
                        xt = xp.tile([P, NLOC], BF16, tag=f"xt{c}",
                                     name=f"xt{c}")
                        nc.scalar.activation(xt[:], zt_tiles[c][:, 0:NLOC],
                                             ACTF.Relu, bias=B[:], scale=A[:])
                        x_tiles.append(xt)

                fused = (HC + 16) <= 512
                ngrp = (NW + WG - 1) // WG
                for g in range(ngrp):
                    j0 = g * WG
                    gw = min(WG, NW - j0)
                    rows = dp.tile([P, WG, ROWE], BF16, tag="rows",
                                   name="rows")[:, 0:gw, :]
                    if ROWE > HC + 16:
                        nc.vector.memset(
                            apx(rows[:, 0, HC + 16:ROWE],
                                [(ROWE, gw), (1, ROWE - HC - 16)]), 0)
                    for jj in range(j0, j0 + gw):
                        psA = pp.tile([P, HC + 16 if fused else HC], F32,
                                      tag="psA", name="psA")
                        psB = None if fused else pp.tile([P, 16], F32,
                                                         tag="psB", name="psB")
                        for c in range(nk):
                            if l == 1:
                                lhs = xT_t[:, jj * P:jj * P + P]
                            else:
                                lhs = x_tiles[c][:, jj * P:(jj + 1) * P]
                            if fused:
                                nc.tensor.matmul(
                                    out=psA[:], lhsT=lhs,
                                    rhs=wc_t[li][:, c * WCW:(c + 1) * WCW],
                                    start=(c == 0), stop=(c == nk - 1))
                            else:
                                rhsW = wc_t[li][:, c * WCW:c * WCW + HC]
                                rhsb = wc_t[li][:, c * WCW + HC:(c + 1) * WCW]
                                nc.tensor.matmul(out=psA[:], lhsT=lhs,
                                                 rhs=rhsW, start=(c == 0),
                                                 stop=(c == nk - 1))
                                nc.tensor.matmul(out=psB[:], lhsT=lhs,
                                                 rhs=rhsb, start=(c == 0),
                                                 stop=(c == nk - 1))
                        ji = jj - j0
                        if fused:
                            nc.scalar.copy(rows[:, ji, 0:HC + 16], psA[:])
                            nc.scalar.copy(aldall[:, jj, :],
                                           psA[:, HC + 8:HC + 16])
                        else:
                            nc.scalar.copy(rows[:, ji, 0:HC], psA[:])
                            nc.scalar.copy(rows[:, ji, HC:HC + 16], psB[:])
                            nc.scalar.copy(aldall[:, jj, :], psB[:, 8:16])
                    dst_ap = bass.AP(ag_in[li], j0 * P * ROWE,
                                     [[ROWE, P], [P * ROWE, gw], [1, ROWE]])
                    nc.sync.dma_start(dst_ap, rows[:])
                nc.gpsimd.collective_compute(
                    "AllGather", ALU.bypass, replica_groups=RG,
                    ins=[ag_in[li].ap().opt()],
                    outs=[tables[li].ap().opt()])

        def edge_phase(l):
            li = l - 1
            HC, C, ROWE = HCS[li], CS[li], ROWES[li]
            ntile = NTILES[li]
            fused = l < 3  # alpha-sum columns ride in the psf matmul
            zt_tiles = []
            if l < 3:
                ztp = tc.tile_pool(name=f"zt{l}", bufs=1)
                ztpool = ztp.__enter__()
                for c in range(ntile):
                    zt_tiles.append(ztpool.tile([P, NLOC], BF16,
                                                tag=f"zT{c}", name=f"zT{c}"))
            with (
                tc.tile_pool(name=f"e{l}", bufs=2 if l == 3 else 3) as ep,
                tc.tile_pool(name=f"eg{l}", bufs=2 if l > 1 else 3) as gp,
                tc.tile_pool(name=f"ez{l}", bufs=1) as zp,
                tc.tile_pool(name=f"eps{l}", bufs=2, space="PSUM") as pp,
                tc.tile_pool(name=f"epf{l}", bufs=2, space="PSUM") as ppf,
            ):
                s12 = [zp.tile([P, 2], F32, tag=f"s12{c}", name=f"s12{c}")
                       for c in range(ntile)]
                for c in range(ntile):
                    nc.vector.memset(s12[c][:], 0.0)
                if l == 3:
                    sgm = [zp.tile([P, 2], F32, tag=f"sgm{c}", name=f"sgm{c}")
                           for c in range(ntile)]
                    for c in range(ntile):
                        nc.vector.memset(sgm[c][:], -3e38)

                for w in range(NW):
                    ns = nsubs[w]
                    base = csum[w]
                    g = gp.tile([P, max(nsubs), ROWE], BF16, tag="g",
                                name="g")[:, 0:ns, :]
                    nc.gpsimd.dma_gather(
                        out_ap=g[:],
                        in_ap=tables[li].ap(),
                        idxs_ap=idx_t[:, base * 8:(base + ns) * 8],
                        num_idxs=ns * P,
                        num_idxs_reg=ns * P,
                        elem_size=ROWE,
                        single_packet=False,
                        queue_num=w % 4,
                    )
                    aldbf = aldall[:, w, :]
                    sel = sel_all[:, base:base + ns, :]
                    selT = ep.tile([P, max(nsubs), P], BF16, tag="selT",
                                   name="selT")[:, 0:ns, :]
                    drep = ep.tile([P, max(nsubs) * P], BF16, tag="drep",
                                   name="drep")[:, 0:ns * P]
                    nc.sync.dma_start(
                        drep[:],
                        dstrep.ap()[:, base * P:(base + ns) * P])
                    # icr[p, s*P+j] = p materialized: all last-dim step 1
                    # so the compare runs in the DVE 2x mode.
                    nc.vector.tensor_tensor(
                        out=selT[:],
                        in0=apx(icr[:, 0:ns * P], [(P, ns), (1, P)]),
                        in1=apx(drep[:], [(P, ns), (1, P)]),
                        op=ALU.is_equal)

                    psew = pp.tile([P, max(nsubs) * 8], F32, tag="psew",
                                   name="psew")[:, 0:ns * 8]
                    for s in range(ns):
                        nc.tensor.matmul(out=psew[:, s * 8:(s + 1) * 8],
                                         lhsT=selT[:, s, :], rhs=aldbf[:],
                                         start=True, stop=True)
                    ew = ep.tile([P, max(nsubs), 8], F32, tag="ew",
                                 name="ew")[:, 0:ns, :]
                    nc.vector.tensor_tensor(
                        out=ew[:],
                        in0=apx(g[:, 0, HC:HC + 8], [(ROWE, ns), (1, 8)]),
                        in1=apx(psew[:], [(8, ns), (1, 8)]),
                        op=ALU.add)
                    ew2 = ep.tile([P, max(nsubs), 8], F32, tag="ew2",
                                  name="ew2")[:, 0:ns, :]
                    nc.vector.tensor_scalar(ew2[:], ew[:], NEG_SLOPE, None,
                                            op0=ALU.mult)
                    nc.vector.tensor_tensor(out=ew2[:], in0=ew[:], in1=ew2[:],
                                            op=ALU.max)
                    wbf = ep.tile([P, max(nsubs), 8], BF16, tag="wbf",
                                  name="wbf")[:, 0:ns, :]
                    nc.scalar.activation(wbf[:], ew2[:], ACTF.Exp)

                    FW = HC + 8 if fused else HC
                    psf = ppf.tile([P, FW], F32, tag="psf", name="psf")
                    psd = None if fused else pp.tile([P, 8], F32, tag="psd",
                                                     name="psd")
                    for s in range(ns):
                        # wh[p, h*C+c] = g[p, h*C+c] * wbf[p, h]; fused:
                        # alpha cols appended so one matmul yields features
                        # + denominators.
                        wh = ep.tile([P, FW], BF16, tag="wh", name="wh")
                        gsl = g[:, s, 0:HC]
                        wsl = wbf[:, s, :]
                        nc.vector.tensor_tensor(
                            out=apx(wh[:, 0:HC], [(C, H), (1, C)]),
                            in0=apx(gsl, [(C, H), (1, C)]),
                            in1=apx(wsl, [(1, H), (0, C)]),
                            op=ALU.mult)
                        if fused:
                            nc.vector.tensor_copy(wh[:, HC:HC + 8], wsl)
                        nc.tensor.matmul(
                            out=psf[:], lhsT=sel[:, s, :], rhs=wh[:],
                            start=(s == 0), stop=(s == ns - 1))
                        if not fused:
                            nc.tensor.matmul(
                                out=psd[:], lhsT=sel[:, s, :], rhs=wsl,
                                start=(s == 0), stop=(s == ns - 1))

                    dsum = psf[:, HC:HC + 8] if fused else psd[:]
                    den = ep.tile([P, 8], F32, tag="den", name="den")
                    nc.vector.tensor_scalar(den[:], dsum, 1e-16, None,
                                            op0=ALU.add)
                    rec = ep.tile([P, 8], F32, tag="rec", name="rec")
                    nc.vector.reciprocal(rec[:], den[:])
                    z = ep.tile([P, HC], F32, tag="z", name="z")
                    # z[p, h*C+c] = psf[p, h*C+c] * rec[p, h]
                    nc.vector.tensor_tensor(
                        out=apx(z[:], [(C, H), (1, C)]),
                        in0=apx(psf[:, 0:HC], [(C, H), (1, C)]),
                        in1=apx(rec[:], [(1, H), (0, C)]),
                        op=ALU.mult)

                    for c in range(ntile):
                        pt = pp.tile([P, P], F32, tag="pt", name="pt")
                        nc.tensor.transpose(pt[:], z[:, c * P:(c + 1) * P],
                                            ident[:])
                        s1w = ep.tile([P, 1], F32, tag="s1w", name="s1w")
                        nc.vector.reduce_sum(s1w[:], pt[:], axis=AX.X)
                        nc.vector.tensor_tensor(
                            out=s12[c][:, 0:1], in0=s12[c][:, 0:1],
                            in1=s1w[:], op=ALU.add)
                        sq = ep.tile([P, P], F32, tag="sq", name="sq")
                        s2w = ep.tile([P, 1], F32, tag="s2w", name="s2w")
                        nc.scalar.activation(sq[:], pt[:], ACTF.Square,
                                             accum_out=s2w[:])
                        nc.vector.tensor_tensor(
                            out=s12[c][:, 1:2], in0=s12[c][:, 1:2],
                            in1=s2w[:], op=ALU.add)
                        if l < 3:
                            nc.scalar.copy(zt_tiles[c][:, w * P:(w + 1) * P],
                                           pt[:])
                        else:
                            nc.tensor.matmul(
                                out=pt[:], lhsT=onesf[:],
                                rhs=pmask_t[:, w * P:(w + 1) * P],
                                start=False, stop=True, skip_group_check=True)
                            wmax = ep.tile([P, 1], F32, tag="wmax",
                                           name="wmax")
                            nc.vector.reduce_max(wmax[:], pt[:], axis=AX.X)
                            seg = 0 if w < SEGW else 1
                            nc.vector.tensor_tensor(
                                out=sgm[c][:, seg:seg + 1],
                                in0=sgm[c][:, seg:seg + 1], in1=wmax[:],
                                op=ALU.max)

                if l < 3:
                    # pack per-core BN stats (hi/lo bf16 pairs) + AllGather
                    pk = ep.tile([P, 4 * ntile], BF16, tag="pk", name="pk")
                    for c in range(ntile):
                        nc.vector.tensor_copy(pk[:, 4 * c:4 * c + 2],
                                              s12[c][:])
                        hif = ep.tile([P, 2], F32, tag="hif", name="hif")
                        nc.vector.tensor_copy(hif[:], pk[:, 4 * c:4 * c + 2])
                        lo = ep.tile([P, 2], F32, tag="lo", name="lo")
                        nc.vector.tensor_tensor(out=lo[:], in0=s12[c][:],
                                                in1=hif[:], op=ALU.subtract)
                        nc.vector.tensor_copy(pk[:, 4 * c + 2:4 * c + 4],
                                              lo[:])
                    nc.sync.dma_start(st_in[li].ap(), pk[:])
                    nc.gpsimd.collective_compute(
                        "AllGather", ALU.bypass, replica_groups=RG,
                        ins=[st_in[li].ap().opt()],
                        outs=[st_out[li].ap().opt()])
                    return zt_tiles, ztp
                # ---------- layer-3 tail ----------
                # one f32 AllGather carries [s1, s2, sgm_seg0, sgm_seg1] per
                # c-tile; every core then derives BN + pooled + FC locally.
                for c in range(ntile):
                    pk = ep.tile([P, 4], F32, tag="pk3", name="pk3")
                    nc.vector.tensor_copy(pk[:, 0:2], s12[c][:])
                    nc.vector.tensor_copy(pk[:, 2:4], sgm[c][:])
                    nc.sync.dma_start(st3_in.ap()[c * P:(c + 1) * P, :],
                                      pk[:])
                nc.gpsimd.collective_compute(
                    "AllGather", ALU.bypass, replica_groups=RG,
                    ins=[st3_in.ap().opt()], outs=[st3_out.ap().opt()])
                with tc.tile_pool(name="tail", bufs=2) as tp:
                    psfc = pp.tile([16, 10], F32, tag="pt", name="psfc")
                    for c in range(4):
                        acc = tp.tile([P, 2], F32, tag="stacc", name="stacc")
                        nc.vector.memset(acc[:], 0.0)
                        sts = []
                        for k in range(NCORES):
                            st = tp.tile([P, 4], F32, tag=f"st{k}",
                                         name=f"st{k}")
                            r0 = (k * 4 + c) * P
                            nc.sync.dma_start(st[:],
                                              st3_out.ap()[r0:r0 + P, :])
                            nc.vector.tensor_tensor(
                                out=acc[:], in0=acc[:], in1=st[:, 0:2],
                                op=ALU.add)
                            sts.append(st)
                        A, B = compute_AB(tp, acc, 2, c)
                        pooled = tp.tile([P, 16], F32, tag="pooled",
                                         name="pooled")
                        for k in range(NCORES):
                            nc.scalar.activation(pooled[:, 2 * k:2 * k + 2],
                                                 sts[k][:, 2:4], ACTF.Relu,
                                                 bias=B[:], scale=A[:])
                        nc.tensor.matmul(out=psfc[:], lhsT=pooled[:],
                                         rhs=fcw_t[c][:], start=(c == 0),
                                         stop=False, skip_group_check=True)
                    nc.tensor.matmul(out=psfc[:], lhsT=ones16[:],
                                     rhs=fcb_t[:], start=False, stop=True,
                                     skip_group_check=True)
                    ot = tp.tile([16, 10], F32, tag="ot", name="ot")
                    nc.vector.tensor_copy(ot[:], psfc[:])
                    nc.sync.dma_start(out.ap(), ot[:])
            return None, None

        for _rep in range(repeat):
            if stages >= 1:
                dense_phase(1, None)
            if stages >= 2:
                zt2, ztp2 = edge_phase(1)
            if stages >= 3:
                dense_phase(2, zt2)
                ztp2.__exit__(None, None, None)
            if stages >= 4:
                zt3, ztp3 = edge_phase(2)
            if stages >= 5:
                dense_phase(3, zt3)
                ztp3.__exit__(None, None, None)
            if stages >= 6:
                edge_phase(3)

    nc.compile()
    return nc


# ================= host preprocessing =================

def prepare(inputs):
    x = np.asarray(inputs["x"], np.float32)
    ei = np.asarray(inputs["edge_index"])
    batch = np.asarray(inputs["batch"]).astype(np.int64)
    N = x.shape[0]
    assert np.all(np.diff(batch) >= 0), "batch must be sorted"
    seg_sizes = np.bincount(batch, minlength=16)
    SEGLEN = int(np.ceil(max(seg_sizes.max(), 1) / P) * P)
    NLOC = 2 * SEGLEN
    NW = NLOC // P
    NPD = 16 * SEGLEN
    assert NPD < 32768, "device node ids must fit int16 for dma_gather"
    seg_start = np.zeros(16, np.int64)
    seg_start[1:] = np.cumsum(seg_sizes)[:-1]
    dev_of = batch * SEGLEN + (np.arange(N) - seg_start[batch])

    src = np.concatenate([ei[0].astype(np.int64), np.arange(N)])
    dst = np.concatenate([ei[1].astype(np.int64), np.arange(N)])
    sdev = dev_of[src]
    ddev = dev_of[dst]
    core = ddev // NLOC
    dloc = ddev % NLOC
    win = dloc // P
    wloc = dloc % P
    key = core * NW + win
    counts = np.bincount(key, minlength=NCORES * NW)
    cw = counts.reshape(NCORES, NW)
    nsubs = np.maximum(1, np.ceil(cw.max(axis=0) / P).astype(np.int64))
    csum = np.zeros(NW + 1, np.int64)
    csum[1:] = np.cumsum(nsubs)
    TS = int(csum[-1])

    perm = np.argsort(key, kind="stable")
    gstart = np.zeros(NCORES * NW, np.int64)
    gstart[1:] = np.cumsum(counts)[:-1]
    pos = np.arange(len(perm)) - gstart[key[perm]]
    kperm = key[perm]
    wbase = (csum[:-1] * P)[kperm % NW]
    slot = (kperm // NW) * (TS * P) + wbase + pos

    src_slot = np.zeros(NCORES * TS * P, np.int16)
    dl_slot = np.full(NCORES * TS * P, 300.0, np.float32)
    src_slot[slot] = sdev[perm].astype(np.int16)
    dl_slot[slot] = wloc[perm].astype(np.float32)
    src_slot = src_slot.reshape(NCORES, TS * P)
    dl_slot = dl_slot.reshape(NCORES, TS * P)

    wcs, bngs, bnbs = [], [], []
    for l, (cin, C) in enumerate([(3, 16), (128, 32), (256, 64)], start=1):
        W = np.asarray(inputs[f"W{l}"], np.float32)
        a_s = np.asarray(inputs[f"as{l}"], np.float32)
        a_d = np.asarray(inputs[f"ad{l}"], np.float32)
        HC = H * C
        Asm = np.zeros((HC, H), np.float32)
        Adm = np.zeros((HC, H), np.float32)
        for hd in range(H):
            Asm[hd * C:(hd + 1) * C, hd] = a_s[hd]
            Adm[hd * C:(hd + 1) * C, hd] = a_d[hd]
        wcat = np.concatenate([W, W @ Asm, W @ Adm], axis=1)  # [cin, HC+16]
        nk = NKS[l - 1]
        if nk > 1:
            wcat = np.concatenate(
                [wcat[c * P:(c + 1) * P] for c in range(nk)], axis=1)
        wcs.append(np.ascontiguousarray(wcat).astype(ml_dtypes.bfloat16))
        nt = HC // P
        bngs.append(np.ascontiguousarray(
            np.asarray(inputs[f"g{l}"], np.float32).reshape(nt, P).T))
        bnbs.append(np.ascontiguousarray(
            np.asarray(inputs[f"be{l}"], np.float32).reshape(nt, P).T))
    fcwb = np.concatenate(
        [np.asarray(inputs["fcW"], np.float32),
         np.asarray(inputs["fcb"], np.float32)[None, :]], axis=0)

    x_dev = np.zeros((NPD, 3), np.float32)
    x_dev[dev_of] = x
    xT_full = np.ascontiguousarray(x_dev.T).astype(ml_dtypes.bfloat16)

    ioro = np.broadcast_to(np.arange(P, dtype=np.float32)[None, :], (P, P))
    ioro = np.ascontiguousarray(ioro).astype(ml_dtypes.bfloat16)
    ioco = np.arange(P, dtype=np.float32)[:, None].astype(ml_dtypes.bfloat16)
    ns_max = int(nsubs.max())
    iocr = np.broadcast_to(
        np.arange(P, dtype=np.float32)[:, None], (P, ns_max * P))
    iocr = np.ascontiguousarray(iocr).astype(ml_dtypes.bfloat16)
    idnt = np.eye(P, dtype=np.float32)

    in_maps = []
    for k in range(NCORES):
        sf = src_slot[k]
        idx_tile = np.zeros((P, TS * 8), np.int16)
        w16 = sf.reshape(TS * 8, 16).T
        idx_tile[:16] = w16
        idx_tile[16:] = np.tile(w16, (7, 1))
        dl = dl_slot[k]
        dloc_cols = np.ascontiguousarray(
            dl.reshape(TS, P).T).astype(ml_dtypes.bfloat16)
        drep = np.broadcast_to(dl.reshape(1, TS * P), (P, TS * P))
        drep = np.ascontiguousarray(drep).astype(ml_dtypes.bfloat16)
        dids = (k * NLOC + np.arange(NW)[None, :] * P
                + np.arange(P)[:, None]).astype(np.int32)
        pm_ = np.zeros((1, NLOC), np.float32)
        for s in (2 * k, 2 * k + 1):
            off = (s - 2 * k) * SEGLEN
            pm_[0, off + seg_sizes[s]: off + SEGLEN] = -1e30
        im = {
            "xT": np.ascontiguousarray(xT_full[:, k * NLOC:(k + 1) * NLOC]),
            "fcwb": fcwb.astype(np.float32),
            "srcs": idx_tile, "dstloc": dloc_cols, "dstrep": drep,
            "dstids": np.ascontiguousarray(dids), "pmask": pm_,
            "ioro": ioro, "ioco": np.ascontiguousarray(ioco),
            "iocr": iocr, "idnt": idnt,
        }
        for l in (1, 2, 3):
            im[f"wc{l}"] = wcs[l - 1]
            im[f"bng{l}"] = bngs[l - 1]
            im[f"bnb{l}"] = bnbs[l - 1]
        in_maps.append(im)
    return SEGLEN, tuple(int(v) for v in nsubs), N, in_maps


_CACHE = {}


def _get_nc(SEGLEN, NSUB, n_real):
    key = (SEGLEN, NSUB, n_real)
    if key not in _CACHE:
        nc = build_gnn(SEGLEN, NSUB, n_real)
        hoist_excess_waits(nc)
        _CACHE[key] = nc
    return _CACHE[key]


def kernel(**inputs):
    patch_walrus()
    SEGLEN, NSUB, n_real, in_maps = prepare(inputs)
    nc = _get_nc(SEGLEN, NSUB, n_real)
    from concourse import bass_utils
    res = bass_utils.run_bass_kernel_spmd(
        nc, in_maps, core_ids=list(range(NCORES)))
    return np.asarray(res.results[0]["out"]).astype(np.float32)
